# revision 20
# baseline (speedup 1.0000x reference)
"""Trainium2 Bass kernel for EuclideanCodebook (VQ-VAE codebook, training fwd).

Strategy (data-parallel over tokens, 8 cores):
  Each core gets a 4096-token shard. On device per core:
    - raw scores 2*s~ = 2*(x @ embed.T) via PE in float32r (fast path,
      ~1e-4 abs error), accumulated over 4 d-chunks in PSUM
    - top-8 candidates per token via DVE max + max_index (first-index ties)
    - quantize rows gathered from the embed table by indirect DMA (top-1 is
      provisional; host may patch after the exact re-check)
    - embed_sum partials via per-tile duplicate-combine (selection-matrix
      matmul, f32r exact for 0/1 weights) + indirect scatter-add DMA into a
      DRAM table (rows K..K+127 are per-partition trash rows so real scatter
      indices stay unique within each DMA)
  Host: candidates are re-ranked with the -0.5*ee adjustment; tokens whose
  adjusted top-2 gap < THETA are re-scored exactly (reference fp32 rounding
  replicated bit-for-bit, jax-CPU xx/ee); tokens violating the outsider
  bound get a full-K exact re-score. Corrections are patched into
  quantize/counts/embed_sum. Then bincount, table sum, EMA + laplace in f32.
"""

import os
import sys

for _p in ("/opt/trn_rl_repo", "/root/.axon_site/_ro/trn_rl_repo"):
    if os.path.isdir(_p) and _p not in sys.path:
        sys.path.insert(0, _p)

import numpy as np

import concourse.bass as bass
import concourse.bacc as bacc
import concourse.mybir as mybir
from concourse.masks import make_identity
from concourse.tile import TileContext

N_CORES = 8
B, T, D, K = 16, 2048, 512, 4096
NTOK = B * T              # 32768
SH = NTOK // N_CORES      # 4096 tokens per core
P = 128
NT = SH // P              # 32 token tiles per core
ND = D // P               # 4 contraction chunks
NQ = 4                    # psum quarters per tile (k-range 1024 each)
KQ = K // NQ              # 1024
EPS = 1e-6
THETA = 2e-3              # host-rescue threshold on adjusted top-2 gap
MARGIN = 4e-3             # outsider-bound margin -> full-K rescore

f32 = mybir.dt.float32
f32r = mybir.dt.float32r
i32 = mybir.dt.int32
u32 = mybir.dt.uint32
Alu = mybir.AluOpType


def build_program():
    nc = bacc.Bacc()

    xT = nc.dram_tensor("xT", [D, SH], f32, kind="ExternalInput")
    xn = nc.dram_tensor("x", [SH, D], f32, kind="ExternalInput")
    eT = nc.dram_tensor("embedT", [D, K], f32, kind="ExternalInput")
    emb = nc.dram_tensor("embed", [K, D], f32, kind="ExternalInput")

    mx_out = nc.dram_tensor("mx_out", [NT, P, 8], f32, kind="ExternalOutput")
    ind_out = nc.dram_tensor("ind_out", [NT, P, 8], u32, kind="ExternalOutput")
    q_out = nc.dram_tensor("q_out", [SH, D], f32, kind="ExternalOutput")
    tab = nc.dram_tensor("tab", [K + P, D], f32, kind="ExternalOutput")

    with TileContext(nc) as tc:
        with (
            tc.tile_pool(name="const", bufs=1) as pc,
            tc.tile_pool(name="tmp", bufs=4) as pt,
            tc.tile_pool(name="wpool", bufs=3) as pw,
            tc.tile_pool(name="xpool", bufs=3) as px,
            tc.tile_pool(name="big", bufs=3) as pb,
            tc.tile_pool(name="small", bufs=5) as ps,
            tc.tile_pool(name="psum_s", bufs=2, space="PSUM") as pp_s,
            tc.tile_pool(name="psum_t", bufs=1, space="PSUM") as pp_t,
            tc.tile_pool(name="psum_a", bufs=1, space="PSUM") as pp_a,
            tc.tile_pool(name="psum_d", bufs=1, space="PSUM") as pp_d,
        ):
            # ---- resident constants ----
            # gpsimd (SWDGE) DMAs: one queue semaphore per transfer, so PE
            # consumers can absorb each with a single LDW sync wait.
            # float32r operands must be produced by a rounding compute op.
            eT_sb = []
            for d in range(ND):
                tile = pc.tile([P, K], f32r, tag=f"eT{d}")
                for h in range(2):
                    raw = pt.tile([P, K // 2], f32, tag="eTraw")
                    nc.gpsimd.dma_start(
                        out=raw[:],
                        in_=eT[d * P:(d + 1) * P, h * (K // 2):(h + 1) * (K // 2)],
                    )
                    nc.vector.tensor_copy(
                        out=tile[:, h * (K // 2):(h + 1) * (K // 2)], in_=raw[:]
                    )
                eT_sb.append(tile)

            ident = pc.tile([P, P], f32, tag="ident")
            make_identity(nc, ident[:])
            # iota_t: every partition row holds [0..127]
            ioti = pc.tile([P, P], i32, tag="ioti")
            nc.gpsimd.iota(out=ioti[:], pattern=[[1, P]], base=0, channel_multiplier=0)
            iota_t = pc.tile([P, P], f32, tag="iotat")
            nc.vector.tensor_copy(out=iota_t[:], in_=ioti[:])
            # per-partition trash-row index K + p and own index p, as f32
            kpi = pc.tile([P, 1], i32, tag="kpi")
            nc.gpsimd.iota(out=kpi[:], pattern=[[1, 1]], base=K, channel_multiplier=1)
            kp = pc.tile([P, 1], f32, tag="kp")
            nc.vector.tensor_copy(out=kp[:], in_=kpi[:])
            p0i = pc.tile([P, 1], i32, tag="p0i")
            nc.gpsimd.iota(out=p0i[:], pattern=[[1, 1]], base=0, channel_multiplier=1)
            p0 = pc.tile([P, 1], f32, tag="p0")
            nc.vector.tensor_copy(out=p0[:], in_=p0i[:])

            # PE wait-absorbers: LDWEIGHTS allows a single sync wait, so make
            # the PE observe each producer semaphore with a tiny matmul whose
            # operands come from exactly one producer. dummy_ps is PE-only.
            dummy_ps = pp_d.tile([P, 16], f32, tag="dummy")

            def absorb(tile_ap):
                m = min(8, tile_ap.shape[1])
                n = min(16, tile_ap.shape[1])
                nc.tensor.matmul(
                    out=dummy_ps[0:m, 0:n], lhsT=tile_ap[:, 0:m],
                    rhs=tile_ap[:, 0:n], start=True, stop=True,
                )

            for d in range(ND):
                absorb(eT_sb[d])
            absorb(ident)

            xT_v = xT[:].rearrange("(d p) n -> p d n", p=P)

            ind_fs = []
            scratch = pc.tile([1, 8], f32, tag="scratch")

            def stage_b1(ind_f):
                # early: PE transpose + ACT copy run while this tile's scans
                # and the next tile's matmuls proceed
                tp_ps = pp_t.tile([P, P], f32, tag="tp")
                nc.tensor.transpose(
                    out=tp_ps[:], in_=ind_f[:].to_broadcast([P, P]), identity=ident[:]
                )
                ind_row = ps.tile([P, P], f32, tag="indrow")
                nc.scalar.copy(out=ind_row[:], in_=tp_ps[:])
                return ind_row

            def stage_b2(ind_f, x_r, ind_row):
                # ---- embed_sum partial: selection-combine + scatter-add ----
                sel = ps.tile([P, P], f32r, tag="sel")
                nc.vector.tensor_tensor(
                    out=sel[:], in0=ind_f[:].to_broadcast([P, P]), in1=ind_row[:],
                    op=Alu.is_equal,
                )
                acc_ps = pp_a.tile([P, D], f32, tag="acc")
                nc.tensor.matmul(
                    out=acc_ps[:], lhsT=sel[:], rhs=x_r[:], start=True, stop=True
                )
                acc_sb = px.tile([P, D], f32, tag="accsb")
                nc.scalar.copy(out=acc_sb[:], in_=acc_ps[:])

                # is_last: p is its group's last occurrence iff
                # max_q(q * sel[p, q]) == p (works for p=0: all-zero row max
                # is 0). Keeps the chain on DVE so the scans never stall on a
                # gpsimd round-trip; the idx' arithmetic below is gpsimd-only
                # and feeds straight into the gpsimd-issued scatter.
                self_sel = sel[:].bitcast(f32)
                selx = ps.tile([P, P], f32, tag="selx")
                nc.vector.tensor_tensor(
                    out=selx[:], in0=self_sel, in1=iota_t[:], op=Alu.mult
                )
                lp8 = ps.tile([P, 8], f32, tag="lp8")
                nc.vector.max(out=lp8[:], in_=selx[:])
                is_last = ps.tile([P, 1], f32, tag="islast")
                nc.vector.tensor_tensor(
                    out=is_last[:], in0=lp8[:, 0:1], in1=p0[:], op=Alu.is_equal
                )
                # idx' = (ind - (K + p)) * is_last + (K + p)
                t1 = ps.tile([P, 1], f32, tag="t1")
                nc.vector.tensor_tensor(
                    out=t1[:], in0=ind_f[:], in1=kp[:], op=Alu.subtract
                )
                t2 = ps.tile([P, 1], f32, tag="t2")
                nc.vector.tensor_tensor(
                    out=t2[:], in0=t1[:], in1=is_last[:], op=Alu.mult
                )
                idxf = ps.tile([P, 1], f32, tag="idxf")
                nc.vector.tensor_tensor(
                    out=idxf[:], in0=t2[:], in1=kp[:], op=Alu.add
                )
                idx2 = ps.tile([P, 1], u32, tag="idx2")
                nc.vector.tensor_copy(out=idx2[:], in_=idxf[:])

                nc.gpsimd.indirect_dma_start(
                    out=tab[:],
                    out_offset=bass.IndirectOffsetOnAxis(ap=idx2[:, 0:1], axis=0),
                    in_=acc_sb[:],
                    in_offset=None,
                    compute_op=Alu.add,
                )

            pending = None  # (ind_f, x_r) of the previous tile

            for t in range(NT):
                if t >= 2:
                    # ACT observes a DVE tick from after max_index(t-2) (the
                    # op that freed this tile's t_sb slot), so the quarter
                    # copies carry only their PE wait.
                    nc.scalar.copy(out=scratch[0:1, 0:1], in_=ind_fs[t - 2][0:1, :])
                if pending is not None:
                    pending = (pending[0], pending[1], stage_b1(pending[0]))

                # ---- load x tiles (one SWDGE DMA each -> one semaphore) ----
                xtraw = pw.tile([P, ND * P], f32, tag="xtraw")
                nc.gpsimd.dma_start(
                    out=xtraw[:].rearrange("p (d c) -> p d c", c=P),
                    in_=xT_v[:, :, t * P:(t + 1) * P],
                )
                xt = pw.tile([P, ND * P], f32r, tag="xt")
                nc.scalar.copy(out=xt[:], in_=xtraw[:])
                x_tile = px.tile([P, D], f32, tag="xtile")
                nc.gpsimd.dma_start(out=x_tile[:], in_=xn[t * P:(t + 1) * P, :])
                x_r = px.tile([P, D], f32r, tag="xr")
                nc.scalar.copy(out=x_r[:], in_=x_tile[:])
                absorb(xt)
                absorb(x_r)

                # ---- raw scores (f32r), weight-reuse d-outer per quarter ----
                t_sb = pb.tile([P, K], f32, tag="tsb")
                for q in range(NQ):
                    psq = pp_s.tile([P, KQ], f32, tag="scores")
                    for d in range(ND):
                        for kk in range(2):
                            k5 = 2 * q + kk
                            nc.tensor.matmul(
                                out=psq[:, kk * 512:(kk + 1) * 512],
                                lhsT=xt[:, d * P:(d + 1) * P],
                                rhs=eT_sb[d][:, k5 * 512:(k5 + 1) * 512],
                                start=(d == 0),
                                stop=(d == ND - 1),
                            )
                    nc.scalar.copy(out=t_sb[:, q * KQ:(q + 1) * KQ], in_=psq[:])

                # ---- top-8 ----
                mx8 = ps.tile([P, 8], f32, tag="mx8")
                ind8 = ps.tile([P, 8], u32, tag="ind8")
                nc.vector.max(out=mx8[:], in_=t_sb[:])
                nc.vector.max_index(out=ind8[:], in_max=mx8[:], in_values=t_sb[:])
                nc.sync.dma_start(out=mx_out[t], in_=mx8[:])
                nc.sync.dma_start(out=ind_out[t], in_=ind8[:])

                # ---- quantize gather (provisional top-1) ----
                quant_sb = px.tile([P, D], f32, tag="quant")
                nc.gpsimd.indirect_dma_start(
                    out=quant_sb[:],
                    out_offset=None,
                    in_=emb[:],
                    in_offset=bass.IndirectOffsetOnAxis(ap=ind8[:, 0:1], axis=0),
                )
                nc.sync.dma_start(out=q_out[t * P:(t + 1) * P, :], in_=quant_sb[:])

                ind_f = ps.tile([P, 1], f32, tag="indf")
                nc.vector.tensor_copy(out=ind_f[:], in_=ind8[:, 0:1])
                ind_fs.append(ind_f)

                # stage B of the previous tile, late half: by now its
                # transpose/ind_row (emitted before this tile's matmuls) are
                # long done, so the DVE ops run without stalling the scans.
                if pending is not None:
                    stage_b2(*pending)
                pending = (ind_f, x_r)

            pending = (pending[0], pending[1], stage_b1(pending[0]))
            stage_b2(*pending)

    nc.compile()
    return nc


_PROGRAM = None


def _get_program():
    global _PROGRAM
    if _PROGRAM is None:
        _PROGRAM = build_program()
    return _PROGRAM


def _ref_stats(xf, embed):
    """xx and ee with the same bits as the jax-CPU reference."""
    try:
        import jax
        import jax.numpy as jnp

        cpu = jax.local_devices(backend="cpu")[0]
        with jax.default_device(cpu):
            xj = jnp.asarray(xf)
            ej = jnp.asarray(embed)
            xx = np.asarray((xj * xj).sum(-1))
            ee = np.asarray((ej * ej).sum(-1))
        return xx, ee
    except Exception:
        xx = np.sum(xf * xf, axis=-1, dtype=np.float32)
        ee = np.sum(embed * embed, axis=-1, dtype=np.float32)
        return xx, ee


def _exact_pick(xf_n, xx_n, ee, embed, cands):
    """Reference-rounding argmin over candidate rows; ties -> lowest k."""
    s = (xf_n[None, :] * embed[cands]).sum(-1, dtype=np.float32)
    d_c = (xx_n - np.float32(2.0) * s) + ee[cands]
    order = np.lexsort((cands, d_c))
    return cands[order[0]]


def kernel(x, embed, cluster_size, embed_avg, decay):
    from concourse.bass_utils import run_bass_kernel_spmd

    x = np.ascontiguousarray(np.asarray(x, dtype=np.float32))
    embed = np.ascontiguousarray(np.asarray(embed, dtype=np.float32))
    cluster_size = np.asarray(cluster_size, dtype=np.float32)
    embed_avg = np.asarray(embed_avg, dtype=np.float32)
    dec = np.float32(np.asarray(decay))

    shape = x.shape
    xf = x.reshape(-1, D)
    xx, ee = _ref_stats(xf, embed)
    embedT = np.ascontiguousarray(embed.T)

    in_maps = []
    for c in range(N_CORES):
        sl = slice(c * SH, (c + 1) * SH)
        xs = np.ascontiguousarray(xf[sl])
        in_maps.append({
            "xT": np.ascontiguousarray(xs.T),
            "x": xs,
            "embedT": embedT,
            "embed": embed,
        })

    nc = _get_program()
    res = run_bass_kernel_spmd(nc, in_maps, list(range(N_CORES))).results

    mx8 = np.concatenate([r["mx_out"].reshape(-1, 8) for r in res])    # s~
    ind8 = np.concatenate([r["ind_out"].reshape(-1, 8) for r in res])
    quantize = np.concatenate([r["q_out"] for r in res], axis=0)
    embed_sum = res[0]["tab"][:K].astype(np.float32)
    for r in res[1:]:
        embed_sum = embed_sum + r["tab"][:K]

    ind = ind8[:, 0].astype(np.int64)

    # ---- host re-rank with the -0.5*ee adjustment ----
    ind8_l = ind8.astype(np.int64)
    adj = mx8.astype(np.float64) - 0.5 * ee.astype(np.float64)[ind8_l]  # [N,8]
    best_c = np.argmax(adj, axis=1)
    adj_sorted = np.sort(adj, axis=1)[:, ::-1]
    gap = adj_sorted[:, 0] - adj_sorted[:, 1]
    # outsider bound: any k outside top-8 has s~ <= mx8[:,7]
    out_bound = mx8[:, 7].astype(np.float64) - 0.5 * float(ee.min())
    need_full = np.nonzero(adj_sorted[:, 0] < out_bound + MARGIN)[0]
    need_cand = np.nonzero(gap < THETA)[0]

    new_ind = ind8_l[np.arange(len(ind)), best_c]
    for n in need_cand:
        new_ind[n] = _exact_pick(xf[n], xx[n], ee, embed, ind8_l[n])
    if len(need_full):
        for n in need_full:
            s = xf[n] @ embedT  # f32 [K]
            d_full = (xx[n] - np.float32(2.0) * s) + ee
            new_ind[n] = int(np.argmin(d_full))

    fix = np.nonzero(new_ind != ind)[0]
    for n in fix:
        old, new = ind[n], new_ind[n]
        quantize[n] = embed[new]
        embed_sum[old] -= xf[n]
        embed_sum[new] += xf[n]
    ind = new_ind

    quantize = quantize.reshape(shape)

    # ---- host finalize (elementwise f32, mirrors reference ops) ----
    one_hot_sum = np.bincount(ind, minlength=K).astype(np.float32)
    one_m_dec = np.float32(1.0) - dec
    new_cluster_size = cluster_size * dec + one_hot_sum * one_m_dec
    new_embed_avg = embed_avg * dec + embed_sum * one_m_dec
    total = np.sum(new_cluster_size, dtype=np.float32)
    smoothed = (new_cluster_size + np.float32(EPS)) / (
        total + np.float32(EPS * K)
    ) * total
    new_embed = new_embed_avg / smoothed[:, None]

    embed_ind = ind.astype(np.int32).reshape(shape[:-1])
    return (quantize, embed_ind, new_cluster_size, new_embed_avg, new_embed)


# revision 21
# speedup vs baseline: 1.0189x; 1.0189x over previous
"""Trainium2 Bass kernel for EuclideanCodebook (VQ-VAE codebook, training fwd).

Strategy (data-parallel over tokens, 8 cores):
  Each core gets a 4096-token shard. On device per core:
    - raw scores 2*s~ = 2*(x @ embed.T) via PE in float32r (fast path,
      ~1e-4 abs error), accumulated over 4 d-chunks in PSUM
    - top-8 candidates per token via DVE max + max_index (first-index ties)
    - quantize rows gathered from the embed table by indirect DMA (top-1 is
      provisional; host may patch after the exact re-check)
    - embed_sum partials via per-tile duplicate-combine (selection-matrix
      matmul, f32r exact for 0/1 weights) + indirect scatter-add DMA into a
      DRAM table (rows K..K+127 are per-partition trash rows so real scatter
      indices stay unique within each DMA)
  Host: candidates are re-ranked with the -0.5*ee adjustment; tokens whose
  adjusted top-2 gap < THETA are re-scored exactly (reference fp32 rounding
  replicated bit-for-bit, jax-CPU xx/ee); tokens violating the outsider
  bound get a full-K exact re-score. Corrections are patched into
  quantize/counts/embed_sum. Then bincount, table sum, EMA + laplace in f32.
"""

import os
import sys

for _p in ("/opt/trn_rl_repo", "/root/.axon_site/_ro/trn_rl_repo"):
    if os.path.isdir(_p) and _p not in sys.path:
        sys.path.insert(0, _p)

import numpy as np

import concourse.bass as bass
import concourse.bacc as bacc
import concourse.mybir as mybir
from concourse.masks import make_identity
from concourse.tile import TileContext

N_CORES = 8
B, T, D, K = 16, 2048, 512, 4096
NTOK = B * T              # 32768
SH = NTOK // N_CORES      # 4096 tokens per core
P = 128
NT = SH // P              # 32 token tiles per core
ND = D // P               # 4 contraction chunks
NQ = 4                    # psum quarters per tile (k-range 1024 each)
KQ = K // NQ              # 1024
EPS = 1e-6
THETA = 2e-3              # host-rescue threshold on adjusted top-2 gap
MARGIN = 4e-3             # outsider-bound margin -> full-K rescore

f32 = mybir.dt.float32
f32r = mybir.dt.float32r
i32 = mybir.dt.int32
u32 = mybir.dt.uint32
Alu = mybir.AluOpType


def build_program():
    nc = bacc.Bacc()

    xT = nc.dram_tensor("xT", [D, SH], f32, kind="ExternalInput")
    xn = nc.dram_tensor("x", [SH, D], f32, kind="ExternalInput")
    eT = nc.dram_tensor("embedT", [D, K], f32, kind="ExternalInput")
    emb = nc.dram_tensor("embed", [K, D], f32, kind="ExternalInput")

    mx_out = nc.dram_tensor("mx_out", [NT, P, 8], f32, kind="ExternalOutput")
    ind_out = nc.dram_tensor("ind_out", [NT, P, 8], u32, kind="ExternalOutput")
    q_out = nc.dram_tensor("q_out", [SH, D], f32, kind="ExternalOutput")
    tab = nc.dram_tensor("tab", [K + P, D], f32, kind="ExternalOutput")

    with TileContext(nc) as tc:
        with (
            tc.tile_pool(name="const", bufs=1) as pc,
            tc.tile_pool(name="tmp", bufs=4) as pt,
            tc.tile_pool(name="wpool", bufs=3) as pw,
            tc.tile_pool(name="xpool", bufs=3) as px,
            tc.tile_pool(name="big", bufs=3) as pb,
            tc.tile_pool(name="small", bufs=5) as ps,
            tc.tile_pool(name="psum_s", bufs=2, space="PSUM") as pp_s,
            tc.tile_pool(name="psum_t", bufs=1, space="PSUM") as pp_t,
            tc.tile_pool(name="psum_a", bufs=1, space="PSUM") as pp_a,
            tc.tile_pool(name="psum_d", bufs=1, space="PSUM") as pp_d,
        ):
            # ---- resident constants ----
            # gpsimd (SWDGE) DMAs: one queue semaphore per transfer, so PE
            # consumers can absorb each with a single LDW sync wait.
            # float32r operands must be produced by a rounding compute op.
            # K-quarter-major load order: the first tile's q=0 matmuls can
            # start once the first four [128,1024] chunks have landed.
            eT_sb = [[None] * NQ for _ in range(ND)]
            for q in range(NQ):
                for d in range(ND):
                    raw = pt.tile([P, KQ], f32, tag="eTraw")
                    nc.gpsimd.dma_start(
                        out=raw[:],
                        in_=eT[d * P:(d + 1) * P, q * KQ:(q + 1) * KQ],
                    )
                    tile = pc.tile([P, KQ], f32r, tag=f"eT{d}_{q}")
                    nc.vector.tensor_copy(out=tile[:], in_=raw[:])
                    eT_sb[d][q] = tile

            ident = pc.tile([P, P], f32, tag="ident")
            make_identity(nc, ident[:])
            # iota_t: every partition row holds [0..127]
            ioti = pc.tile([P, P], i32, tag="ioti")
            nc.gpsimd.iota(out=ioti[:], pattern=[[1, P]], base=0, channel_multiplier=0)
            iota_t = pc.tile([P, P], f32, tag="iotat")
            nc.vector.tensor_copy(out=iota_t[:], in_=ioti[:])
            # per-partition trash-row index K + p and own index p, as f32
            kpi = pc.tile([P, 1], i32, tag="kpi")
            nc.gpsimd.iota(out=kpi[:], pattern=[[1, 1]], base=K, channel_multiplier=1)
            kp = pc.tile([P, 1], f32, tag="kp")
            nc.vector.tensor_copy(out=kp[:], in_=kpi[:])
            p0i = pc.tile([P, 1], i32, tag="p0i")
            nc.gpsimd.iota(out=p0i[:], pattern=[[1, 1]], base=0, channel_multiplier=1)
            p0 = pc.tile([P, 1], f32, tag="p0")
            nc.vector.tensor_copy(out=p0[:], in_=p0i[:])

            # PE wait-absorbers: LDWEIGHTS allows a single sync wait, so make
            # the PE observe each producer semaphore with a tiny matmul whose
            # operands come from exactly one producer. dummy_ps is PE-only.
            dummy_ps = pp_d.tile([P, 16], f32, tag="dummy")

            def absorb(tile_ap):
                m = min(8, tile_ap.shape[1])
                n = min(16, tile_ap.shape[1])
                nc.tensor.matmul(
                    out=dummy_ps[0:m, 0:n], lhsT=tile_ap[:, 0:m],
                    rhs=tile_ap[:, 0:n], start=True, stop=True,
                )

            for d in range(ND):
                for q in range(NQ):
                    absorb(eT_sb[d][q])
            absorb(ident)

            xT_v = xT[:].rearrange("(d p) n -> p d n", p=P)

            ind_fs = []
            scratch = pc.tile([1, 8], f32, tag="scratch")

            def stage_b1(ind_f):
                # early: PE transpose + ACT copy run while this tile's scans
                # and the next tile's matmuls proceed
                tp_ps = pp_t.tile([P, P], f32, tag="tp")
                nc.tensor.transpose(
                    out=tp_ps[:], in_=ind_f[:].to_broadcast([P, P]), identity=ident[:]
                )
                ind_row = ps.tile([P, P], f32, tag="indrow")
                nc.scalar.copy(out=ind_row[:], in_=tp_ps[:])
                return ind_row

            def stage_b2(ind_f, x_r, ind_row):
                # ---- embed_sum partial: selection-combine + scatter-add ----
                sel = ps.tile([P, P], f32r, tag="sel")
                nc.vector.tensor_tensor(
                    out=sel[:], in0=ind_f[:].to_broadcast([P, P]), in1=ind_row[:],
                    op=Alu.is_equal,
                )
                acc_ps = pp_a.tile([P, D], f32, tag="acc")
                nc.tensor.matmul(
                    out=acc_ps[:], lhsT=sel[:], rhs=x_r[:], start=True, stop=True
                )
                acc_sb = px.tile([P, D], f32, tag="accsb")
                nc.scalar.copy(out=acc_sb[:], in_=acc_ps[:])

                # is_last: p is its group's last occurrence iff
                # max_q(q * sel[p, q]) == p (works for p=0: all-zero row max
                # is 0). Keeps the chain on DVE so the scans never stall on a
                # gpsimd round-trip; the idx' arithmetic below is gpsimd-only
                # and feeds straight into the gpsimd-issued scatter.
                self_sel = sel[:].bitcast(f32)
                selx = ps.tile([P, P], f32, tag="selx")
                nc.vector.tensor_tensor(
                    out=selx[:], in0=self_sel, in1=iota_t[:], op=Alu.mult
                )
                lp8 = ps.tile([P, 8], f32, tag="lp8")
                nc.vector.max(out=lp8[:], in_=selx[:])
                is_last = ps.tile([P, 1], f32, tag="islast")
                nc.vector.tensor_tensor(
                    out=is_last[:], in0=lp8[:, 0:1], in1=p0[:], op=Alu.is_equal
                )
                # idx' = (ind - (K + p)) * is_last + (K + p)
                t1 = ps.tile([P, 1], f32, tag="t1")
                nc.gpsimd.tensor_tensor(
                    out=t1[:], in0=ind_f[:], in1=kp[:], op=Alu.subtract
                )
                t2 = ps.tile([P, 1], f32, tag="t2")
                nc.gpsimd.tensor_tensor(
                    out=t2[:], in0=t1[:], in1=is_last[:], op=Alu.mult
                )
                idxf = ps.tile([P, 1], f32, tag="idxf")
                nc.gpsimd.tensor_tensor(
                    out=idxf[:], in0=t2[:], in1=kp[:], op=Alu.add
                )
                idx2 = ps.tile([P, 1], u32, tag="idx2")
                nc.gpsimd.tensor_copy(out=idx2[:], in_=idxf[:])

                nc.gpsimd.indirect_dma_start(
                    out=tab[:],
                    out_offset=bass.IndirectOffsetOnAxis(ap=idx2[:, 0:1], axis=0),
                    in_=acc_sb[:],
                    in_offset=None,
                    compute_op=Alu.add,
                )

            pending = None  # (ind_f, x_r) of the previous tile

            for t in range(NT):
                if t >= 2:
                    # ACT observes a DVE tick from after max_index(t-2) (the
                    # op that freed this tile's t_sb slot), so the quarter
                    # copies carry only their PE wait.
                    nc.scalar.copy(out=scratch[0:1, 0:1], in_=ind_fs[t - 2][0:1, :])
                if pending is not None:
                    pending = (pending[0], pending[1], stage_b1(pending[0]))

                # ---- load x tiles (one SWDGE DMA each -> one semaphore) ----
                xtraw = pw.tile([P, ND * P], f32, tag="xtraw")
                nc.gpsimd.dma_start(
                    out=xtraw[:].rearrange("p (d c) -> p d c", c=P),
                    in_=xT_v[:, :, t * P:(t + 1) * P],
                )
                xt = pw.tile([P, ND * P], f32r, tag="xt")
                nc.scalar.copy(out=xt[:], in_=xtraw[:])
                x_tile = px.tile([P, D], f32, tag="xtile")
                nc.gpsimd.dma_start(out=x_tile[:], in_=xn[t * P:(t + 1) * P, :])
                x_r = px.tile([P, D], f32r, tag="xr")
                nc.scalar.copy(out=x_r[:], in_=x_tile[:])
                absorb(xt)
                absorb(x_r)

                # ---- raw scores (f32r), weight-reuse d-outer per quarter ----
                t_sb = pb.tile([P, K], f32, tag="tsb")
                for q in range(NQ):
                    psq = pp_s.tile([P, KQ], f32, tag="scores")
                    for d in range(ND):
                        for kk in range(2):
                            k5 = 2 * q + kk
                            nc.tensor.matmul(
                                out=psq[:, kk * 512:(kk + 1) * 512],
                                lhsT=xt[:, d * P:(d + 1) * P],
                                rhs=eT_sb[d][q][:, kk * 512:(kk + 1) * 512],
                                start=(d == 0),
                                stop=(d == ND - 1),
                            )
                    nc.scalar.copy(out=t_sb[:, q * KQ:(q + 1) * KQ], in_=psq[:])

                # ---- top-8 ----
                mx8 = ps.tile([P, 8], f32, tag="mx8")
                ind8 = ps.tile([P, 8], u32, tag="ind8")
                nc.vector.max(out=mx8[:], in_=t_sb[:])
                nc.vector.max_index(out=ind8[:], in_max=mx8[:], in_values=t_sb[:])
                nc.sync.dma_start(out=mx_out[t], in_=mx8[:])
                nc.sync.dma_start(out=ind_out[t], in_=ind8[:])

                # ---- quantize gather (provisional top-1) ----
                quant_sb = px.tile([P, D], f32, tag="quant")
                nc.gpsimd.indirect_dma_start(
                    out=quant_sb[:],
                    out_offset=None,
                    in_=emb[:],
                    in_offset=bass.IndirectOffsetOnAxis(ap=ind8[:, 0:1], axis=0),
                )
                nc.sync.dma_start(out=q_out[t * P:(t + 1) * P, :], in_=quant_sb[:])

                ind_f = ps.tile([P, 1], f32, tag="indf")
                nc.vector.tensor_copy(out=ind_f[:], in_=ind8[:, 0:1])
                ind_fs.append(ind_f)

                # stage B of the previous tile, late half: by now its
                # transpose/ind_row (emitted before this tile's matmuls) are
                # long done, so the DVE ops run without stalling the scans.
                if pending is not None:
                    stage_b2(*pending)
                pending = (ind_f, x_r)

            pending = (pending[0], pending[1], stage_b1(pending[0]))
            stage_b2(*pending)

    nc.compile()
    return nc


_PROGRAM = None


def _get_program():
    global _PROGRAM
    if _PROGRAM is None:
        _PROGRAM = build_program()
    return _PROGRAM


def _ref_stats(xf, embed):
    """xx and ee with the same bits as the jax-CPU reference."""
    try:
        import jax
        import jax.numpy as jnp

        cpu = jax.local_devices(backend="cpu")[0]
        with jax.default_device(cpu):
            xj = jnp.asarray(xf)
            ej = jnp.asarray(embed)
            xx = np.asarray((xj * xj).sum(-1))
            ee = np.asarray((ej * ej).sum(-1))
        return xx, ee
    except Exception:
        xx = np.sum(xf * xf, axis=-1, dtype=np.float32)
        ee = np.sum(embed * embed, axis=-1, dtype=np.float32)
        return xx, ee


def _exact_pick(xf_n, xx_n, ee, embed, cands):
    """Reference-rounding argmin over candidate rows; ties -> lowest k."""
    s = (xf_n[None, :] * embed[cands]).sum(-1, dtype=np.float32)
    d_c = (xx_n - np.float32(2.0) * s) + ee[cands]
    order = np.lexsort((cands, d_c))
    return cands[order[0]]


def kernel(x, embed, cluster_size, embed_avg, decay):
    from concourse.bass_utils import run_bass_kernel_spmd

    x = np.ascontiguousarray(np.asarray(x, dtype=np.float32))
    embed = np.ascontiguousarray(np.asarray(embed, dtype=np.float32))
    cluster_size = np.asarray(cluster_size, dtype=np.float32)
    embed_avg = np.asarray(embed_avg, dtype=np.float32)
    dec = np.float32(np.asarray(decay))

    shape = x.shape
    xf = x.reshape(-1, D)
    xx, ee = _ref_stats(xf, embed)
    embedT = np.ascontiguousarray(embed.T)

    in_maps = []
    for c in range(N_CORES):
        sl = slice(c * SH, (c + 1) * SH)
        xs = np.ascontiguousarray(xf[sl])
        in_maps.append({
            "xT": np.ascontiguousarray(xs.T),
            "x": xs,
            "embedT": embedT,
            "embed": embed,
        })

    nc = _get_program()
    res = run_bass_kernel_spmd(nc, in_maps, list(range(N_CORES))).results

    mx8 = np.concatenate([r["mx_out"].reshape(-1, 8) for r in res])    # s~
    ind8 = np.concatenate([r["ind_out"].reshape(-1, 8) for r in res])
    quantize = np.concatenate([r["q_out"] for r in res], axis=0)
    embed_sum = res[0]["tab"][:K].astype(np.float32)
    for r in res[1:]:
        embed_sum = embed_sum + r["tab"][:K]

    ind = ind8[:, 0].astype(np.int64)

    # ---- host re-rank with the -0.5*ee adjustment ----
    ind8_l = ind8.astype(np.int64)
    adj = mx8.astype(np.float64) - 0.5 * ee.astype(np.float64)[ind8_l]  # [N,8]
    best_c = np.argmax(adj, axis=1)
    adj_sorted = np.sort(adj, axis=1)[:, ::-1]
    gap = adj_sorted[:, 0] - adj_sorted[:, 1]
    # outsider bound: any k outside top-8 has s~ <= mx8[:,7]
    out_bound = mx8[:, 7].astype(np.float64) - 0.5 * float(ee.min())
    need_full = np.nonzero(adj_sorted[:, 0] < out_bound + MARGIN)[0]
    need_cand = np.nonzero(gap < THETA)[0]

    new_ind = ind8_l[np.arange(len(ind)), best_c]
    for n in need_cand:
        new_ind[n] = _exact_pick(xf[n], xx[n], ee, embed, ind8_l[n])
    if len(need_full):
        for n in need_full:
            s = xf[n] @ embedT  # f32 [K]
            d_full = (xx[n] - np.float32(2.0) * s) + ee
            new_ind[n] = int(np.argmin(d_full))

    fix = np.nonzero(new_ind != ind)[0]
    for n in fix:
        old, new = ind[n], new_ind[n]
        quantize[n] = embed[new]
        embed_sum[old] -= xf[n]
        embed_sum[new] += xf[n]
    ind = new_ind

    quantize = quantize.reshape(shape)

    # ---- host finalize (elementwise f32, mirrors reference ops) ----
    one_hot_sum = np.bincount(ind, minlength=K).astype(np.float32)
    one_m_dec = np.float32(1.0) - dec
    new_cluster_size = cluster_size * dec + one_hot_sum * one_m_dec
    new_embed_avg = embed_avg * dec + embed_sum * one_m_dec
    total = np.sum(new_cluster_size, dtype=np.float32)
    smoothed = (new_cluster_size + np.float32(EPS)) / (
        total + np.float32(EPS * K)
    ) * total
    new_embed = new_embed_avg / smoothed[:, None]

    embed_ind = ind.astype(np.int32).reshape(shape[:-1])
    return (quantize, embed_ind, new_cluster_size, new_embed_avg, new_embed)


# revision 24
# speedup vs baseline: 1.0532x; 1.0336x over previous
"""Trainium2 Bass kernel for EuclideanCodebook (VQ-VAE codebook, training fwd).

Strategy (data-parallel over tokens, 8 cores):
  Each core gets a 4096-token shard. On device per core:
    - raw scores 2*s~ = 2*(x @ embed.T) via PE in float32r (fast path,
      ~1e-4 abs error), accumulated over 4 d-chunks in PSUM
    - top-8 candidates per token via DVE max + max_index (first-index ties)
    - quantize rows gathered from the embed table by indirect DMA (top-1 is
      provisional; host may patch after the exact re-check)
    - embed_sum partials via per-tile duplicate-combine (selection-matrix
      matmul, f32r exact for 0/1 weights) + indirect scatter-add DMA into a
      DRAM table (rows K..K+127 are per-partition trash rows so real scatter
      indices stay unique within each DMA)
  Host: candidates are re-ranked with the -0.5*ee adjustment; tokens whose
  adjusted top-2 gap < THETA are re-scored exactly (reference fp32 rounding
  replicated bit-for-bit, jax-CPU xx/ee); tokens violating the outsider
  bound get a full-K exact re-score. Corrections are patched into
  quantize/counts/embed_sum. Then bincount, table sum, EMA + laplace in f32.
"""

import os
import sys

for _p in ("/opt/trn_rl_repo", "/root/.axon_site/_ro/trn_rl_repo"):
    if os.path.isdir(_p) and _p not in sys.path:
        sys.path.insert(0, _p)

import numpy as np

import concourse.bass as bass
import concourse.bacc as bacc
import concourse.mybir as mybir
from concourse.masks import make_identity
from concourse.tile import TileContext

N_CORES = 8
B, T, D, K = 16, 2048, 512, 4096
NTOK = B * T              # 32768
SH = NTOK // N_CORES      # 4096 tokens per core
P = 128
NT = SH // P              # 32 token tiles per core
ND = D // P               # 4 contraction chunks
NQ = 4                    # psum quarters per tile (k-range 1024 each)
KQ = K // NQ              # 1024
EPS = 1e-6
THETA = 2e-3              # host-rescue threshold on adjusted top-2 gap
MARGIN = 4e-3             # outsider-bound margin -> full-K rescore

f32 = mybir.dt.float32
f32r = mybir.dt.float32r
i32 = mybir.dt.int32
u32 = mybir.dt.uint32
Alu = mybir.AluOpType


def build_program():
    nc = bacc.Bacc()

    xT = nc.dram_tensor("xT", [D, SH], f32, kind="ExternalInput")
    xn = nc.dram_tensor("x", [SH, D], f32, kind="ExternalInput")
    eT = nc.dram_tensor("embedT", [D, K], f32, kind="ExternalInput")
    emb = nc.dram_tensor("embed", [K, D], f32, kind="ExternalInput")

    mx_out = nc.dram_tensor("mx_out", [NT, P, 8], f32, kind="ExternalOutput")
    ind_out = nc.dram_tensor("ind_out", [NT, P, 8], u32, kind="ExternalOutput")
    q_out = nc.dram_tensor("q_out", [SH, D], f32, kind="ExternalOutput")
    tab = nc.dram_tensor("tab", [K + P, D], f32, kind="ExternalOutput")

    with TileContext(nc) as tc:
        with (
            tc.tile_pool(name="const", bufs=1) as pc,
            tc.tile_pool(name="tmp", bufs=2) as pt,
            tc.tile_pool(name="wpool", bufs=3) as pw,
            tc.tile_pool(name="xpool", bufs=4) as px,
            tc.tile_pool(name="big", bufs=3) as pb,
            tc.tile_pool(name="small", bufs=8) as ps,
            tc.tile_pool(name="psum_s", bufs=2, space="PSUM") as pp_s,
            tc.tile_pool(name="psum_t", bufs=1, space="PSUM") as pp_t,
            tc.tile_pool(name="psum_a", bufs=1, space="PSUM") as pp_a,
            tc.tile_pool(name="psum_d", bufs=1, space="PSUM") as pp_d,
        ):
            # ---- resident constants ----
            # gpsimd (SWDGE) DMAs: one queue semaphore per transfer, so PE
            # consumers can absorb each with a single LDW sync wait.
            # float32r operands must be produced by a rounding compute op.
            eT_sb = []
            for d in range(ND):
                raw = pt.tile([P, K], f32, tag="eTraw")
                nc.sync.dma_start(out=raw[:], in_=eT[d * P:(d + 1) * P, :])
                tile = pc.tile([P, K], f32r, tag=f"eT{d}")
                nc.vector.tensor_copy(out=tile[:], in_=raw[:])
                eT_sb.append(tile)

            ident = pc.tile([P, P], f32, tag="ident")
            make_identity(nc, ident[:])
            ut = pc.tile([P, P], f32, tag="ut")
            nc.gpsimd.memset(ut[:], 0.0)
            nc.gpsimd.affine_select(
                out=ut[:], in_=ut[:], compare_op=Alu.is_ge, fill=1.0,
                base=0, pattern=[[-1, P]], channel_multiplier=1,
            )
            # per-partition trash-row index K + p, as f32
            kpi = pc.tile([P, 1], i32, tag="kpi")
            nc.gpsimd.iota(out=kpi[:], pattern=[[1, 1]], base=K, channel_multiplier=1)
            kp = pc.tile([P, 1], f32, tag="kp")
            nc.vector.tensor_copy(out=kp[:], in_=kpi[:])

            # PE wait-absorbers: LDWEIGHTS allows a single sync wait, so make
            # the PE observe each producer semaphore with a tiny matmul whose
            # operands come from exactly one producer. dummy_ps is PE-only.
            dummy_ps = pp_d.tile([P, 16], f32, tag="dummy")

            def absorb(tile_ap):
                m = min(8, tile_ap.shape[1])
                n = min(16, tile_ap.shape[1])
                nc.tensor.matmul(
                    out=dummy_ps[0:m, 0:n], lhsT=tile_ap[:, 0:m],
                    rhs=tile_ap[:, 0:n], start=True, stop=True,
                )

            for d in range(ND):
                absorb(eT_sb[d])
            absorb(ident)

            xT_v = xT[:].rearrange("(d p) n -> p d n", p=P)

            ind_fs = []
            scratch = pc.tile([1, 8], f32, tag="scratch")

            def stage_b(ind_f, x_r):
                # ---- embed_sum partial: selection-combine + scatter-add ----
                tp_ps = pp_t.tile([P, P], f32, tag="tp")
                nc.tensor.transpose(
                    out=tp_ps[:], in_=ind_f[:].to_broadcast([P, P]), identity=ident[:]
                )
                ind_row = ps.tile([P, P], f32, tag="indrow")
                nc.scalar.copy(out=ind_row[:], in_=tp_ps[:])
                sel = ps.tile([P, P], f32r, tag="sel")
                nc.vector.tensor_tensor(
                    out=sel[:], in0=ind_f[:].to_broadcast([P, P]), in1=ind_row[:],
                    op=Alu.is_equal,
                )
                acc_ps = pp_a.tile([P, D], f32, tag="acc")
                nc.tensor.matmul(
                    out=acc_ps[:], lhsT=sel[:], rhs=x_r[:], start=True, stop=True
                )
                acc_sb = px.tile([P, D], f32, tag="accsb")
                nc.scalar.copy(out=acc_sb[:], in_=acc_ps[:])

                # is_last: no later duplicate in tile -> unique scatter target;
                # duplicates are redirected to per-partition trash rows
                self_sel = sel[:].bitcast(f32)
                sel_ut = ps.tile([P, P], f32, tag="selut")
                nc.gpsimd.tensor_tensor(
                    out=sel_ut[:], in0=self_sel, in1=ut[:], op=Alu.mult
                )
                cnt = ps.tile([P, 1], f32, tag="cnt")
                nc.vector.reduce_sum(
                    out=cnt[:], in_=sel_ut[:], axis=mybir.AxisListType.X
                )
                is_last = ps.tile([P, 1], f32, tag="islast")
                nc.gpsimd.tensor_scalar(
                    out=is_last[:], in0=cnt[:], scalar1=0.0, scalar2=None,
                    op0=Alu.is_equal,
                )
                # idx' = (ind - (K + p)) * is_last + (K + p)
                t1 = ps.tile([P, 1], f32, tag="t1")
                nc.gpsimd.tensor_tensor(
                    out=t1[:], in0=ind_f[:], in1=kp[:], op=Alu.subtract
                )
                t2 = ps.tile([P, 1], f32, tag="t2")
                nc.gpsimd.tensor_tensor(
                    out=t2[:], in0=t1[:], in1=is_last[:], op=Alu.mult
                )
                idxf = ps.tile([P, 1], f32, tag="idxf")
                nc.gpsimd.tensor_tensor(
                    out=idxf[:], in0=t2[:], in1=kp[:], op=Alu.add
                )
                idx2 = ps.tile([P, 1], u32, tag="idx2")
                nc.gpsimd.tensor_copy(out=idx2[:], in_=idxf[:])

                nc.gpsimd.indirect_dma_start(
                    out=tab[:],
                    out_offset=bass.IndirectOffsetOnAxis(ap=idx2[:, 0:1], axis=0),
                    in_=acc_sb[:],
                    in_offset=None,
                    compute_op=Alu.add,
                )

            pending = None  # (ind_f, x_r) of the previous tile

            for t in range(NT):
                if t >= 2:
                    # ACT observes a DVE tick from after max_index(t-2) (the
                    # op that freed this tile's t_sb slot), so the quarter
                    # copies carry only their PE wait.
                    nc.scalar.copy(out=scratch[0:1, 0:1], in_=ind_fs[t - 2][0:1, :])
                # ---- load x tiles (one SWDGE DMA each -> one semaphore) ----
                xtraw = pw.tile([P, ND * P], f32, tag="xtraw")
                nc.gpsimd.dma_start(
                    out=xtraw[:].rearrange("p (d c) -> p d c", c=P),
                    in_=xT_v[:, :, t * P:(t + 1) * P],
                )
                xt = pw.tile([P, ND * P], f32r, tag="xt")
                nc.scalar.copy(out=xt[:], in_=xtraw[:])
                x_tile = px.tile([P, D], f32, tag="xtile")
                nc.gpsimd.dma_start(out=x_tile[:], in_=xn[t * P:(t + 1) * P, :])
                x_r = px.tile([P, D], f32r, tag="xr")
                nc.scalar.copy(out=x_r[:], in_=x_tile[:])
                absorb(xt)
                absorb(x_r)

                # ---- raw scores (f32r), weight-reuse d-outer per quarter ----
                t_sb = pb.tile([P, K], f32, tag="tsb")
                for q in range(NQ):
                    psq = pp_s.tile([P, KQ], f32, tag="scores")
                    for d in range(ND):
                        for kk in range(2):
                            k5 = 2 * q + kk
                            nc.tensor.matmul(
                                out=psq[:, kk * 512:(kk + 1) * 512],
                                lhsT=xt[:, d * P:(d + 1) * P],
                                rhs=eT_sb[d][:, k5 * 512:(k5 + 1) * 512],
                                start=(d == 0),
                                stop=(d == ND - 1),
                            )
                    nc.scalar.copy(out=t_sb[:, q * KQ:(q + 1) * KQ], in_=psq[:])

                # ---- top-8 ----
                mx8 = ps.tile([P, 8], f32, tag="mx8")
                ind8 = ps.tile([P, 8], u32, tag="ind8")
                nc.vector.max(out=mx8[:], in_=t_sb[:])
                nc.vector.max_index(out=ind8[:], in_max=mx8[:], in_values=t_sb[:])
                nc.sync.dma_start(out=mx_out[t], in_=mx8[:])
                nc.sync.dma_start(out=ind_out[t], in_=ind8[:])

                # ---- quantize gather (provisional top-1) ----
                quant_sb = px.tile([P, D], f32, tag="quant")
                nc.gpsimd.indirect_dma_start(
                    out=quant_sb[:],
                    out_offset=None,
                    in_=emb[:],
                    in_offset=bass.IndirectOffsetOnAxis(ap=ind8[:, 0:1], axis=0),
                )
                nc.sync.dma_start(out=q_out[t * P:(t + 1) * P, :], in_=quant_sb[:])

                ind_f = ps.tile([P, 1], f32, tag="indf")
                nc.vector.tensor_copy(out=ind_f[:], in_=ind8[:, 0:1])
                ind_fs.append(ind_f)

                # stage B of the previous tile: its cross-engine chain is long
                # since resolved, so the DVE/PE ops run without stalling the
                # current tile's scans (software pipelining by one tile).
                if pending is not None:
                    stage_b(*pending)
                pending = (ind_f, x_r)

            stage_b(*pending)

    nc.compile()
    return nc


_PROGRAM = None


def _get_program():
    global _PROGRAM
    if _PROGRAM is None:
        _PROGRAM = build_program()
    return _PROGRAM


def _ref_stats(xf, embed):
    """xx and ee with the same bits as the jax-CPU reference."""
    try:
        import jax
        import jax.numpy as jnp

        cpu = jax.local_devices(backend="cpu")[0]
        with jax.default_device(cpu):
            xj = jnp.asarray(xf)
            ej = jnp.asarray(embed)
            xx = np.asarray((xj * xj).sum(-1))
            ee = np.asarray((ej * ej).sum(-1))
        return xx, ee
    except Exception:
        xx = np.sum(xf * xf, axis=-1, dtype=np.float32)
        ee = np.sum(embed * embed, axis=-1, dtype=np.float32)
        return xx, ee


def _exact_pick(xf_n, xx_n, ee, embed, cands):
    """Reference-rounding argmin over candidate rows; ties -> lowest k."""
    s = (xf_n[None, :] * embed[cands]).sum(-1, dtype=np.float32)
    d_c = (xx_n - np.float32(2.0) * s) + ee[cands]
    order = np.lexsort((cands, d_c))
    return cands[order[0]]


def kernel(x, embed, cluster_size, embed_avg, decay):
    from concourse.bass_utils import run_bass_kernel_spmd

    x = np.ascontiguousarray(np.asarray(x, dtype=np.float32))
    embed = np.ascontiguousarray(np.asarray(embed, dtype=np.float32))
    cluster_size = np.asarray(cluster_size, dtype=np.float32)
    embed_avg = np.asarray(embed_avg, dtype=np.float32)
    dec = np.float32(np.asarray(decay))

    shape = x.shape
    xf = x.reshape(-1, D)
    xx, ee = _ref_stats(xf, embed)
    embedT = np.ascontiguousarray(embed.T)

    in_maps = []
    for c in range(N_CORES):
        sl = slice(c * SH, (c + 1) * SH)
        xs = np.ascontiguousarray(xf[sl])
        in_maps.append({
            "xT": np.ascontiguousarray(xs.T),
            "x": xs,
            "embedT": embedT,
            "embed": embed,
        })

    nc = _get_program()
    res = run_bass_kernel_spmd(nc, in_maps, list(range(N_CORES))).results

    mx8 = np.concatenate([r["mx_out"].reshape(-1, 8) for r in res])    # s~
    ind8 = np.concatenate([r["ind_out"].reshape(-1, 8) for r in res])
    quantize = np.concatenate([r["q_out"] for r in res], axis=0)
    embed_sum = res[0]["tab"][:K].astype(np.float32)
    for r in res[1:]:
        embed_sum = embed_sum + r["tab"][:K]

    ind = ind8[:, 0].astype(np.int64)

    # ---- host re-rank with the -0.5*ee adjustment ----
    ind8_l = ind8.astype(np.int64)
    adj = mx8.astype(np.float64) - 0.5 * ee.astype(np.float64)[ind8_l]  # [N,8]
    best_c = np.argmax(adj, axis=1)
    adj_sorted = np.sort(adj, axis=1)[:, ::-1]
    gap = adj_sorted[:, 0] - adj_sorted[:, 1]
    # outsider bound: any k outside top-8 has s~ <= mx8[:,7]
    out_bound = mx8[:, 7].astype(np.float64) - 0.5 * float(ee.min())
    need_full = np.nonzero(adj_sorted[:, 0] < out_bound + MARGIN)[0]
    need_cand = np.nonzero(gap < THETA)[0]

    new_ind = ind8_l[np.arange(len(ind)), best_c]
    for n in need_cand:
        new_ind[n] = _exact_pick(xf[n], xx[n], ee, embed, ind8_l[n])
    if len(need_full):
        for n in need_full:
            s = xf[n] @ embedT  # f32 [K]
            d_full = (xx[n] - np.float32(2.0) * s) + ee
            new_ind[n] = int(np.argmin(d_full))

    fix = np.nonzero(new_ind != ind)[0]
    for n in fix:
        old, new = ind[n], new_ind[n]
        quantize[n] = embed[new]
        embed_sum[old] -= xf[n]
        embed_sum[new] += xf[n]
    ind = new_ind

    quantize = quantize.reshape(shape)

    # ---- host finalize (elementwise f32, mirrors reference ops) ----
    one_hot_sum = np.bincount(ind, minlength=K).astype(np.float32)
    one_m_dec = np.float32(1.0) - dec
    new_cluster_size = cluster_size * dec + one_hot_sum * one_m_dec
    new_embed_avg = embed_avg * dec + embed_sum * one_m_dec
    total = np.sum(new_cluster_size, dtype=np.float32)
    smoothed = (new_cluster_size + np.float32(EPS)) / (
        total + np.float32(EPS * K)
    ) * total
    new_embed = new_embed_avg / smoothed[:, None]

    embed_ind = ind.astype(np.int32).reshape(shape[:-1])
    return (quantize, embed_ind, new_cluster_size, new_embed_avg, new_embed)


# revision 25
# speedup vs baseline: 1.0754x; 1.0211x over previous
"""Trainium2 Bass kernel for EuclideanCodebook (VQ-VAE codebook, training fwd).

Strategy (data-parallel over tokens, 8 cores):
  Each core gets a 4096-token shard. On device per core:
    - raw scores 2*s~ = 2*(x @ embed.T) via PE in float32r (fast path,
      ~1e-4 abs error), accumulated over 4 d-chunks in PSUM
    - top-8 candidates per token via DVE max + max_index (first-index ties)
    - quantize rows gathered from the embed table by indirect DMA (top-1 is
      provisional; host may patch after the exact re-check)
    - embed_sum partials via per-tile duplicate-combine (selection-matrix
      matmul, f32r exact for 0/1 weights) + indirect scatter-add DMA into a
      DRAM table (rows K..K+127 are per-partition trash rows so real scatter
      indices stay unique within each DMA)
  Host: candidates are re-ranked with the -0.5*ee adjustment; tokens whose
  adjusted top-2 gap < THETA are re-scored exactly (reference fp32 rounding
  replicated bit-for-bit, jax-CPU xx/ee); tokens violating the outsider
  bound get a full-K exact re-score. Corrections are patched into
  quantize/counts/embed_sum. Then bincount, table sum, EMA + laplace in f32.
"""

import os
import sys

for _p in ("/opt/trn_rl_repo", "/root/.axon_site/_ro/trn_rl_repo"):
    if os.path.isdir(_p) and _p not in sys.path:
        sys.path.insert(0, _p)

import numpy as np

import concourse.bass as bass
import concourse.bacc as bacc
import concourse.mybir as mybir
from concourse.masks import make_identity
from concourse.tile import TileContext

N_CORES = 8
B, T, D, K = 16, 2048, 512, 4096
NTOK = B * T              # 32768
SH = NTOK // N_CORES      # 4096 tokens per core
P = 128
NT = SH // P              # 32 token tiles per core
ND = D // P               # 4 contraction chunks
NQ = 4                    # psum quarters per tile (k-range 1024 each)
KQ = K // NQ              # 1024
EPS = 1e-6
THETA = 2e-3              # host-rescue threshold on adjusted top-2 gap
MARGIN = 4e-3             # outsider-bound margin -> full-K rescore

f32 = mybir.dt.float32
f32r = mybir.dt.float32r
i32 = mybir.dt.int32
u32 = mybir.dt.uint32
Alu = mybir.AluOpType


def build_program():
    nc = bacc.Bacc()

    xT = nc.dram_tensor("xT", [D, SH], f32, kind="ExternalInput")
    xn = nc.dram_tensor("x", [SH, D], f32, kind="ExternalInput")
    eT = nc.dram_tensor("embedT", [D, K], f32, kind="ExternalInput")
    emb = nc.dram_tensor("embed", [K, D], f32, kind="ExternalInput")

    mx_out = nc.dram_tensor("mx_out", [NT, P, 8], f32, kind="ExternalOutput")
    ind_out = nc.dram_tensor("ind_out", [NT, P, 8], u32, kind="ExternalOutput")
    q_out = nc.dram_tensor("q_out", [SH, D], f32, kind="ExternalOutput")
    tab = nc.dram_tensor("tab", [K + P, D], f32, kind="ExternalOutput")

    with TileContext(nc) as tc:
        with (
            tc.tile_pool(name="const", bufs=1) as pc,
            tc.tile_pool(name="tmp", bufs=4) as pt,
            tc.tile_pool(name="wpool", bufs=3) as pw,
            tc.tile_pool(name="xpool", bufs=4) as px,
            tc.tile_pool(name="big", bufs=3) as pb,
            tc.tile_pool(name="small", bufs=8) as ps,
            tc.tile_pool(name="psum_s", bufs=2, space="PSUM") as pp_s,
            tc.tile_pool(name="psum_t", bufs=1, space="PSUM") as pp_t,
            tc.tile_pool(name="psum_a", bufs=1, space="PSUM") as pp_a,
            tc.tile_pool(name="psum_d", bufs=1, space="PSUM") as pp_d,
        ):
            # ---- resident constants ----
            # gpsimd (SWDGE) DMAs: one queue semaphore per transfer, so PE
            # consumers can absorb each with a single LDW sync wait.
            # float32r operands must be produced by a rounding compute op.
            eT_sb = []
            for d in range(ND):
                tile = pc.tile([P, K], f32r, tag=f"eT{d}")
                for h in range(2):
                    # alternate HWDGE/SWDGE paths to double load bandwidth
                    raw = pt.tile([P, K // 2], f32, tag="eTraw")
                    eng = nc.sync if (2 * d + h) % 2 == 0 else nc.gpsimd
                    eng.dma_start(
                        out=raw[:],
                        in_=eT[d * P:(d + 1) * P, h * (K // 2):(h + 1) * (K // 2)],
                    )
                    nc.vector.tensor_copy(
                        out=tile[:, h * (K // 2):(h + 1) * (K // 2)], in_=raw[:]
                    )
                eT_sb.append(tile)

            ident = pc.tile([P, P], f32, tag="ident")
            make_identity(nc, ident[:])
            ut = pc.tile([P, P], f32, tag="ut")
            nc.gpsimd.memset(ut[:], 0.0)
            nc.gpsimd.affine_select(
                out=ut[:], in_=ut[:], compare_op=Alu.is_ge, fill=1.0,
                base=0, pattern=[[-1, P]], channel_multiplier=1,
            )
            # per-partition trash-row index K + p, as f32
            kpi = pc.tile([P, 1], i32, tag="kpi")
            nc.gpsimd.iota(out=kpi[:], pattern=[[1, 1]], base=K, channel_multiplier=1)
            kp = pc.tile([P, 1], f32, tag="kp")
            nc.vector.tensor_copy(out=kp[:], in_=kpi[:])

            # PE wait-absorbers: LDWEIGHTS allows a single sync wait, so make
            # the PE observe each producer semaphore with a tiny matmul whose
            # operands come from exactly one producer. dummy_ps is PE-only.
            dummy_ps = pp_d.tile([P, 16], f32, tag="dummy")

            def absorb(tile_ap):
                m = min(8, tile_ap.shape[1])
                n = min(16, tile_ap.shape[1])
                nc.tensor.matmul(
                    out=dummy_ps[0:m, 0:n], lhsT=tile_ap[:, 0:m],
                    rhs=tile_ap[:, 0:n], start=True, stop=True,
                )

            for d in range(ND):
                absorb(eT_sb[d])
            absorb(ident)

            xT_v = xT[:].rearrange("(d p) n -> p d n", p=P)

            ind_fs = []
            scratch = pc.tile([1, 8], f32, tag="scratch")

            def stage_b(ind_f, x_r):
                # ---- embed_sum partial: selection-combine + scatter-add ----
                tp_ps = pp_t.tile([P, P], f32, tag="tp")
                nc.tensor.transpose(
                    out=tp_ps[:], in_=ind_f[:].to_broadcast([P, P]), identity=ident[:]
                )
                ind_row = ps.tile([P, P], f32, tag="indrow")
                nc.scalar.copy(out=ind_row[:], in_=tp_ps[:])
                sel = ps.tile([P, P], f32r, tag="sel")
                nc.vector.tensor_tensor(
                    out=sel[:], in0=ind_f[:].to_broadcast([P, P]), in1=ind_row[:],
                    op=Alu.is_equal,
                )
                acc_ps = pp_a.tile([P, D], f32, tag="acc")
                nc.tensor.matmul(
                    out=acc_ps[:], lhsT=sel[:], rhs=x_r[:], start=True, stop=True
                )
                acc_sb = px.tile([P, D], f32, tag="accsb")
                nc.scalar.copy(out=acc_sb[:], in_=acc_ps[:])

                # is_last: no later duplicate in tile -> unique scatter target;
                # duplicates are redirected to per-partition trash rows
                self_sel = sel[:].bitcast(f32)
                sel_ut = ps.tile([P, P], f32, tag="selut")
                nc.gpsimd.tensor_tensor(
                    out=sel_ut[:], in0=self_sel, in1=ut[:], op=Alu.mult
                )
                cnt = ps.tile([P, 1], f32, tag="cnt")
                nc.vector.reduce_sum(
                    out=cnt[:], in_=sel_ut[:], axis=mybir.AxisListType.X
                )
                is_last = ps.tile([P, 1], f32, tag="islast")
                nc.gpsimd.tensor_scalar(
                    out=is_last[:], in0=cnt[:], scalar1=0.0, scalar2=None,
                    op0=Alu.is_equal,
                )
                # idx' = (ind - (K + p)) * is_last + (K + p)
                t1 = ps.tile([P, 1], f32, tag="t1")
                nc.gpsimd.tensor_tensor(
                    out=t1[:], in0=ind_f[:], in1=kp[:], op=Alu.subtract
                )
                t2 = ps.tile([P, 1], f32, tag="t2")
                nc.gpsimd.tensor_tensor(
                    out=t2[:], in0=t1[:], in1=is_last[:], op=Alu.mult
                )
                idxf = ps.tile([P, 1], f32, tag="idxf")
                nc.gpsimd.tensor_tensor(
                    out=idxf[:], in0=t2[:], in1=kp[:], op=Alu.add
                )
                idx2 = ps.tile([P, 1], u32, tag="idx2")
                nc.gpsimd.tensor_copy(out=idx2[:], in_=idxf[:])

                nc.gpsimd.indirect_dma_start(
                    out=tab[:],
                    out_offset=bass.IndirectOffsetOnAxis(ap=idx2[:, 0:1], axis=0),
                    in_=acc_sb[:],
                    in_offset=None,
                    compute_op=Alu.add,
                )

            pending = None  # (ind_f, x_r) of the previous tile

            for t in range(NT):
                if t >= 2:
                    # ACT observes a DVE tick from after max_index(t-2) (the
                    # op that freed this tile's t_sb slot), so the quarter
                    # copies carry only their PE wait.
                    nc.scalar.copy(out=scratch[0:1, 0:1], in_=ind_fs[t - 2][0:1, :])
                # ---- load x tiles (one SWDGE DMA each -> one semaphore) ----
                xtraw = pw.tile([P, ND * P], f32, tag="xtraw")
                nc.gpsimd.dma_start(
                    out=xtraw[:].rearrange("p (d c) -> p d c", c=P),
                    in_=xT_v[:, :, t * P:(t + 1) * P],
                )
                xt = pw.tile([P, ND * P], f32r, tag="xt")
                nc.scalar.copy(out=xt[:], in_=xtraw[:])
                x_tile = px.tile([P, D], f32, tag="xtile")
                nc.gpsimd.dma_start(out=x_tile[:], in_=xn[t * P:(t + 1) * P, :])
                x_r = px.tile([P, D], f32r, tag="xr")
                nc.scalar.copy(out=x_r[:], in_=x_tile[:])
                absorb(xt)
                absorb(x_r)

                # ---- raw scores (f32r), weight-reuse d-outer per quarter ----
                t_sb = pb.tile([P, K], f32, tag="tsb")
                for q in range(NQ):
                    psq = pp_s.tile([P, KQ], f32, tag="scores")
                    for d in range(ND):
                        for kk in range(2):
                            k5 = 2 * q + kk
                            nc.tensor.matmul(
                                out=psq[:, kk * 512:(kk + 1) * 512],
                                lhsT=xt[:, d * P:(d + 1) * P],
                                rhs=eT_sb[d][:, k5 * 512:(k5 + 1) * 512],
                                start=(d == 0),
                                stop=(d == ND - 1),
                            )
                    nc.scalar.copy(out=t_sb[:, q * KQ:(q + 1) * KQ], in_=psq[:])

                # ---- top-8 ----
                mx8 = ps.tile([P, 8], f32, tag="mx8")
                ind8 = ps.tile([P, 8], u32, tag="ind8")
                nc.vector.max(out=mx8[:], in_=t_sb[:])
                nc.vector.max_index(out=ind8[:], in_max=mx8[:], in_values=t_sb[:])
                nc.sync.dma_start(out=mx_out[t], in_=mx8[:])
                nc.sync.dma_start(out=ind_out[t], in_=ind8[:])

                # ---- quantize gather (provisional top-1) ----
                quant_sb = px.tile([P, D], f32, tag="quant")
                nc.gpsimd.indirect_dma_start(
                    out=quant_sb[:],
                    out_offset=None,
                    in_=emb[:],
                    in_offset=bass.IndirectOffsetOnAxis(ap=ind8[:, 0:1], axis=0),
                )
                nc.sync.dma_start(out=q_out[t * P:(t + 1) * P, :], in_=quant_sb[:])

                ind_f = ps.tile([P, 1], f32, tag="indf")
                nc.vector.tensor_copy(out=ind_f[:], in_=ind8[:, 0:1])
                ind_fs.append(ind_f)

                # stage B of the previous tile: its cross-engine chain is long
                # since resolved, so the DVE/PE ops run without stalling the
                # current tile's scans (software pipelining by one tile).
                if pending is not None:
                    stage_b(*pending)
                pending = (ind_f, x_r)

            stage_b(*pending)

    nc.compile()
    return nc


_PROGRAM = None


def _get_program():
    global _PROGRAM
    if _PROGRAM is None:
        _PROGRAM = build_program()
    return _PROGRAM


def _ref_stats(xf, embed):
    """xx and ee with the same bits as the jax-CPU reference."""
    try:
        import jax
        import jax.numpy as jnp

        cpu = jax.local_devices(backend="cpu")[0]
        with jax.default_device(cpu):
            xj = jnp.asarray(xf)
            ej = jnp.asarray(embed)
            xx = np.asarray((xj * xj).sum(-1))
            ee = np.asarray((ej * ej).sum(-1))
        return xx, ee
    except Exception:
        xx = np.sum(xf * xf, axis=-1, dtype=np.float32)
        ee = np.sum(embed * embed, axis=-1, dtype=np.float32)
        return xx, ee


def _exact_pick(xf_n, xx_n, ee, embed, cands):
    """Reference-rounding argmin over candidate rows; ties -> lowest k."""
    s = (xf_n[None, :] * embed[cands]).sum(-1, dtype=np.float32)
    d_c = (xx_n - np.float32(2.0) * s) + ee[cands]
    order = np.lexsort((cands, d_c))
    return cands[order[0]]


def kernel(x, embed, cluster_size, embed_avg, decay):
    from concourse.bass_utils import run_bass_kernel_spmd

    x = np.ascontiguousarray(np.asarray(x, dtype=np.float32))
    embed = np.ascontiguousarray(np.asarray(embed, dtype=np.float32))
    cluster_size = np.asarray(cluster_size, dtype=np.float32)
    embed_avg = np.asarray(embed_avg, dtype=np.float32)
    dec = np.float32(np.asarray(decay))

    shape = x.shape
    xf = x.reshape(-1, D)
    xx, ee = _ref_stats(xf, embed)
    embedT = np.ascontiguousarray(embed.T)

    in_maps = []
    for c in range(N_CORES):
        sl = slice(c * SH, (c + 1) * SH)
        xs = np.ascontiguousarray(xf[sl])
        in_maps.append({
            "xT": np.ascontiguousarray(xs.T),
            "x": xs,
            "embedT": embedT,
            "embed": embed,
        })

    nc = _get_program()
    res = run_bass_kernel_spmd(nc, in_maps, list(range(N_CORES))).results

    mx8 = np.concatenate([r["mx_out"].reshape(-1, 8) for r in res])    # s~
    ind8 = np.concatenate([r["ind_out"].reshape(-1, 8) for r in res])
    quantize = np.concatenate([r["q_out"] for r in res], axis=0)
    embed_sum = res[0]["tab"][:K].astype(np.float32)
    for r in res[1:]:
        embed_sum = embed_sum + r["tab"][:K]

    ind = ind8[:, 0].astype(np.int64)

    # ---- host re-rank with the -0.5*ee adjustment ----
    ind8_l = ind8.astype(np.int64)
    adj = mx8.astype(np.float64) - 0.5 * ee.astype(np.float64)[ind8_l]  # [N,8]
    best_c = np.argmax(adj, axis=1)
    adj_sorted = np.sort(adj, axis=1)[:, ::-1]
    gap = adj_sorted[:, 0] - adj_sorted[:, 1]
    # outsider bound: any k outside top-8 has s~ <= mx8[:,7]
    out_bound = mx8[:, 7].astype(np.float64) - 0.5 * float(ee.min())
    need_full = np.nonzero(adj_sorted[:, 0] < out_bound + MARGIN)[0]
    need_cand = np.nonzero(gap < THETA)[0]

    new_ind = ind8_l[np.arange(len(ind)), best_c]
    for n in need_cand:
        new_ind[n] = _exact_pick(xf[n], xx[n], ee, embed, ind8_l[n])
    if len(need_full):
        for n in need_full:
            s = xf[n] @ embedT  # f32 [K]
            d_full = (xx[n] - np.float32(2.0) * s) + ee
            new_ind[n] = int(np.argmin(d_full))

    fix = np.nonzero(new_ind != ind)[0]
    for n in fix:
        old, new = ind[n], new_ind[n]
        quantize[n] = embed[new]
        embed_sum[old] -= xf[n]
        embed_sum[new] += xf[n]
    ind = new_ind

    quantize = quantize.reshape(shape)

    # ---- host finalize (elementwise f32, mirrors reference ops) ----
    one_hot_sum = np.bincount(ind, minlength=K).astype(np.float32)
    one_m_dec = np.float32(1.0) - dec
    new_cluster_size = cluster_size * dec + one_hot_sum * one_m_dec
    new_embed_avg = embed_avg * dec + embed_sum * one_m_dec
    total = np.sum(new_cluster_size, dtype=np.float32)
    smoothed = (new_cluster_size + np.float32(EPS)) / (
        total + np.float32(EPS * K)
    ) * total
    new_embed = new_embed_avg / smoothed[:, None]

    embed_ind = ind.astype(np.int32).reshape(shape[:-1])
    return (quantize, embed_ind, new_cluster_size, new_embed_avg, new_embed)
